# revision 1
# baseline (speedup 1.0000x reference)
"""Self-contained Trainium2 Bass kernel for the "Attentive" GNN message-passing
problem:

    x: [8192, 256] f32, attn_vectors: [4, 256] f32
    e_h = l2_normalize(attn_vectors[h] * x, axis=-1)        # [H, N, D]
    Y   = concat_h(e_h)                                     # [N, H*D]
    out = (Y @ Y.T) / H                                     # [N, N]

Strategy (8 NeuronCores, SPMD, no collectives):
  - Output rows are sharded 8 x 1024; every core receives the FULL x plus its
    own x_local row-shard as separate inputs, so the program is core-agnostic.
  - Key algebra: out[i,j] = sum_k (x*a^2*r/H)[i,k] * (x*r)[j,k] with
    r_h[n] = 1/sqrt(max(sum_d (a_h[d]*x[n,d])^2, eps)); a^2 and the 1/H are
    folded into the (small, resident) lhsT side only, so the streamed rhs
    panels need just one elementwise multiply each.
  - Everything runs in "features on partitions" layout (x^T), obtained by
    staging a bf16 copy of x in DRAM (per-panel tiles, fine-grained deps)
    and reading it back through the DMA xbar transpose.
  - Row norms are computed as transposed PE matmuls xsq^T @ a^2 so the
    max/sqrt/reciprocal chain runs in [128, 16] layout (all DVE lanes
    active); rnorm rows bounce through DRAM and come back as one batched
    broadcast DMA per panel (step-0 partition APs are legal on DRAM).
  - Matmul inputs are bf16 (PE runs f32 at quarter rate); PSUM accumulates
    f32; each panel's 8 PSUM tiles are copied into one SBUF tile and leave
    in a single 2 MB DMA.
  - DMAs are batched aggressively: the SP sequencer pays ~600 ns per
    dma_start, so the panel pipeline uses ~7 DMAs per 512-column panel.
"""

from contextlib import ExitStack

import numpy as np

N, D, H = 8192, 256, 4
NCORES = 8
NLOC = N // NCORES  # 1024 output rows per core
P = 128
PANEL = 512
NPANELS = N // PANEL  # 16
RBLK = NLOC // P  # 8 row blocks of the local output
KCH = (H * D) // P  # 8 contraction chunks of 128
CHD = D // P  # 2 chunks per head
SUB = PANEL // P  # 4 column sub-blocks per panel
EPS = 1e-12

_COMPILED = {}


def _build_bass():
    import concourse.bass as bass
    import concourse.tile as tile
    from concourse import bacc, mybir

    f32 = mybir.dt.float32
    bf16 = mybir.dt.bfloat16

    nc = bacc.Bacc(
        "TRN2",
        target_bir_lowering=False,
        debug=False,
        enable_asserts=False,
        num_devices=NCORES,
    )
    x_t = nc.dram_tensor("x", [N, D], f32, kind="ExternalInput")
    xl_t = nc.dram_tensor("x_local", [NLOC, D], f32, kind="ExternalInput")
    # Host-precomputed functions of attn_vectors (tiny):
    #   w_sq[d, c*4+h]  = attn[h, c*128+d]^2          (bf16, norm matmul rhs)
    #   asq[d, kc]      = 0.25*attn[h, c*128+d]^2     (f32, kc = h*2+c)
    ws_t = nc.dram_tensor("w_sq", [P, CHD * H], bf16, kind="ExternalInput")
    aq_t = nc.dram_tensor("asq", [P, KCH], f32, kind="ExternalInput")
    out_t = nc.dram_tensor("out", [NLOC, N], f32, kind="ExternalOutput")

    x, xl, out = x_t.ap(), xl_t.ap(), out_t.ap()

    with tile.TileContext(nc) as tc, ExitStack() as ctx:
        consts = ctx.enter_context(tc.tile_pool(name="consts", bufs=1))
        loads = ctx.enter_context(tc.tile_pool(name="loads", bufs=6))
        xtp = ctx.enter_context(tc.tile_pool(name="xtp", bufs=1))
        sq = ctx.enter_context(tc.tile_pool(name="sq", bufs=4))
        small = ctx.enter_context(tc.tile_pool(name="small", bufs=3))
        bcp = ctx.enter_context(tc.tile_pool(name="bcp", bufs=3))
        rhsp = ctx.enter_context(tc.tile_pool(name="rhsp", bufs=3))
        outp = ctx.enter_context(tc.tile_pool(name="outp", bufs=2))
        dram = ctx.enter_context(tc.tile_pool(name="dram", bufs=1, space="DRAM"))
        ps_norm = ctx.enter_context(
            tc.tile_pool(name="ps_norm", bufs=2, space="PSUM")
        )
        ps_out = ctx.enter_context(
            tc.tile_pool(name="ps_out", bufs=4, space="PSUM")
        )

        from concourse.masks import make_identity

        w_sq = consts.tile([P, CHD * H], bf16)
        nc.sync.dma_start(w_sq[:], ws_t.ap()[:])
        asq = consts.tile([P, KCH], f32)
        nc.sync.dma_start(asq[:], aq_t.ap()[:])
        ident = consts.tile([P, P], f32)
        make_identity(nc, ident[:])
        identb = consts.tile([P, P], bf16)
        make_identity(nc, identb[:])

        def sb_rearr(tile_ap):
            return tile_ap[:].rearrange("q (i d) -> q i d", i=SUB)

        def x_rearr(ap, row0):
            return ap[row0 : row0 + PANEL, :].rearrange(
                "(i q) d -> q i d", q=P
            )

        def prepass(src_ap, row0, xT_tile, name):
            """Load 512 source rows (one batched DMA), transpose them on the
            PE into bf16 x^T, and park this panel's rnorm in DRAM.
            The PSUM->SBUF copy after each transpose doubles as the f32->bf16
            cast."""
            xt = loads.tile([P, SUB * D], f32, tag="xload")
            nc.sync.dma_start(sb_rearr(xt), x_rearr(src_ap, row0))
            # Round to bf16 before the PE transpose: bf16 streams the PE at
            # 1 cycle/row vs 2 for f32, and the rounding happens exactly once
            # either way (the PSUM->SBUF copy used to do it).
            xtb = loads.tile([P, SUB * D], bf16, tag="xtb")
            nc.vector.tensor_copy(xtb[:], xt[:])
            for c in range(CHD):
                tp4 = ps_norm.tile([P, PANEL], bf16, tag="tp")
                for i in range(SUB):
                    nc.tensor.transpose(
                        tp4[:, i * P : (i + 1) * P],
                        xtb[:, i * D + c * P : i * D + (c + 1) * P],
                        identb[:],
                    )
                nc.vector.tensor_copy(
                    xT_tile[:, c * PANEL : (c + 1) * PANEL], tp4[:]
                )
            pn = ps_norm.tile([P, SUB * H], f32, tag="pn")
            xsq = sq.tile([P, CHD * PANEL], bf16, tag="xsq")
            nc.vector.tensor_mul(xsq[:], xT_tile[:], xT_tile[:])
            for i in range(SUB):
                for c in range(CHD):
                    nc.tensor.matmul(
                        pn[:, i * H : (i + 1) * H],
                        xsq[:, c * PANEL + i * P : c * PANEL + (i + 1) * P],
                        w_sq[:, c * H : (c + 1) * H],
                        start=(c == 0),
                        stop=(c == CHD - 1),
                    )
            # eps-clamp; the input AP also permutes [q,(i h)] -> [q,(h i)]
            # so that after the PE transpose the store is contiguous.
            clamped = small.tile([P, SUB * H], f32, tag="clamped")
            nc.vector.tensor_scalar_max(
                clamped[:],
                pn[:].rearrange("q (i h) -> q h i", h=H),
                EPS,
            )
            root = small.tile([P, SUB * H], f32, tag="root")
            nc.scalar.sqrt(root[:], clamped[:])
            rnorm = small.tile([P, SUB * H], f32, tag="rnorm")
            nc.vector.reciprocal(rnorm[:], root[:])
            # [128, 16] -> [16, 128]; row j = h*4+i, so the flat DRAM tile
            # is rnorm_h[i*128+q] at offset h*512 + i*128 + q (h-major).
            pt = ps_norm.tile([SUB * H, P], f32, tag="tp")
            nc.tensor.transpose(pt[:], rnorm[:], ident[:])
            rno = small.tile([SUB * H, P], f32, tag="rno")
            nc.vector.tensor_copy(rno[:], pt[:])
            rnd = dram.tile([SUB * H, P], f32, name=name)
            nc.sync.dma_start(rnd[:], rno[:])
            return rnd

        def bcast_rnorm(rnd):
            """[128, 4*512] f32: bc[:, h*512 + n] = rnorm_h[n], one DMA."""
            bc = bcp.tile([P, H * PANEL], f32, tag="bc")
            src = bass.AP(
                rnd.tensor,
                rnd.offset,
                [[0, P], [PANEL, H], [1, PANEL]],
            )
            nc.sync.dma_start(
                bc[:].rearrange("p (h n) -> p h n", h=H), src
            )
            return bc

        # ---- all prepasses first ------------------------------------------
        # Tile's per-engine instruction order is static, so the lhsT-build
        # DVE ops (which wait on the rnorm DRAM bounce) must come AFTER every
        # prepass op or they head-of-line-block the prepass copies and starve
        # the PE of transpose work during the wait.
        lhsT = consts.tile([P, KCH * NLOC], bf16)
        xlocT = []
        lrnds = []
        for lp in range(2):
            t = consts.tile([P, CHD * PANEL], bf16, name=f"xlocT{lp}")
            xlocT.append(t)
            lrnds.append(prepass(xl, lp * PANEL, t, f"lrnd{lp}"))
        PIPE = 4  # panels of prepass lookahead over the main loop
        xTs = []
        rnds = []

        def prepass_x(p):
            t = xtp.tile([P, CHD * PANEL], bf16, name=f"xT{p}")
            xTs.append(t)
            rnds.append(prepass(x, p * PANEL, t, f"rnd{p}"))

        for p in range(PIPE):
            prepass_x(p)

        # ---- resident lhsT -------------------------------------------------
        for lp in range(2):
            t = xlocT[lp]
            bc = bcast_rnorm(lrnds[lp])
            for h in range(H):
                for c in range(CHD):
                    kc = h * CHD + c
                    scaled = sq.tile([P, PANEL], f32, tag="scaled")
                    nc.vector.tensor_scalar_mul(
                        scaled[:],
                        bc[:, h * PANEL : (h + 1) * PANEL],
                        asq[:, kc : kc + 1],
                    )
                    nc.vector.tensor_mul(
                        lhsT[
                            :,
                            kc * NLOC + lp * PANEL : kc * NLOC + (lp + 1) * PANEL,
                        ],
                        t[:, c * PANEL : (c + 1) * PANEL],
                        scaled[:],
                    )

        # ---- main loop over 16 column panels (prepass pipelined ahead) -----
        for p in range(NPANELS):
            bc = bcast_rnorm(rnds[p])
            # Issue the prepass for panel p+PIPE after this panel's broadcast:
            # its DVE/PE ops fill scheduling gaps without ever blocking the
            # current panel's work (static per-engine order).
            rhs = rhsp.tile([P, KCH * PANEL], bf16, tag="rhs")
            # One batched multiply builds the whole Y'^T panel:
            #   rhs[:, (h*2+c)*512 + n] = xT[:, c*512 + n] * bc[:, h*512 + n]
            xT = xTs[p]
            in0 = bass.AP(
                xT.tensor,
                xT.offset,
                [list(xT.ap[0]), [0, H], [PANEL, CHD], [1, PANEL]],
            )
            in1 = bass.AP(
                bc.tensor,
                bc.offset,
                [list(bc.ap[0]), [PANEL, H], [0, CHD], [1, PANEL]],
            )
            nc.vector.tensor_tensor(
                rhs[:].rearrange("q (h c n) -> q h c n", h=H, c=CHD),
                in0,
                in1,
                mybir.AluOpType.mult,
            )
            if p + PIPE < NPANELS:
                prepass_x(p + PIPE)

            ot = outp.tile([P, RBLK * PANEL], f32, tag="ot")
            for r in range(RBLK):
                acc = ps_out.tile([P, PANEL], f32, tag="acc")
                for kc in range(KCH):
                    nc.tensor.matmul(
                        acc[:],
                        lhsT[:, kc * NLOC + r * P : kc * NLOC + (r + 1) * P],
                        rhs[:, kc * PANEL : (kc + 1) * PANEL],
                        start=(kc == 0),
                        stop=(kc == KCH - 1),
                    )
                nc.vector.tensor_copy(
                    ot[:, r * PANEL : (r + 1) * PANEL], acc[:]
                )
                # Last panel: ship each row block as soon as it is ready so
                # the kernel tail is one small DMA, not copy-all-then-DMA.
                if p == NPANELS - 1:
                    nc.sync.dma_start(
                        out[
                            r * P : (r + 1) * P,
                            p * PANEL : (p + 1) * PANEL,
                        ],
                        ot[:, r * PANEL : (r + 1) * PANEL],
                    )
            if p != NPANELS - 1:
                nc.sync.dma_start(
                    out[:, p * PANEL : (p + 1) * PANEL].rearrange(
                        "(r q) c -> q r c", q=P
                    ),
                    ot[:].rearrange("q (r c) -> q r c", r=RBLK),
                )

    nc.compile()
    return nc


def _get_compiled():
    if "nc" not in _COMPILED:
        _COMPILED["nc"] = _build_bass()
    return _COMPILED["nc"]


def host_side_inputs(x, attn):
    """Per-core input maps (w_sq / asq are tiny host-precomputed functions
    of attn_vectors; see _build_bass)."""
    import ml_dtypes

    w_sq = np.zeros((P, CHD * H), dtype=np.float32)
    asq = np.zeros((P, KCH), dtype=np.float32)
    for c in range(CHD):
        w_sq[:, c * H : (c + 1) * H] = (attn[:, c * P : (c + 1) * P] ** 2).T
    for kc in range(KCH):
        h, c = divmod(kc, CHD)
        asq[:, kc] = 0.25 * attn[h, c * P : (c + 1) * P] ** 2
    w_sq = w_sq.astype(ml_dtypes.bfloat16)
    return [
        {
            "x": x,
            "x_local": np.ascontiguousarray(x[c * NLOC : (c + 1) * NLOC]),
            "w_sq": w_sq,
            "asq": asq,
        }
        for c in range(NCORES)
    ]


def kernel(**inputs) -> np.ndarray:
    from concourse import bass_utils

    x = np.ascontiguousarray(np.asarray(inputs["x"], dtype=np.float32))
    attn = np.ascontiguousarray(
        np.asarray(inputs["attn_vectors"], dtype=np.float32)
    )
    nc = _get_compiled()
    res = bass_utils.run_bass_kernel_spmd(
        nc, host_side_inputs(x, attn), core_ids=list(range(NCORES))
    )
    out = np.concatenate([r["out"] for r in res.results], axis=0)
    # The exact result is symmetric; the bf16 rounding errors of the two
    # triangles are independent, so symmetrizing averages them down.
    return ((out + out.T) * 0.5).astype(np.float32)



# revision 2
# speedup vs baseline: 1.4482x; 1.4482x over previous
"""Self-contained Trainium2 Bass kernel for the "Attentive" GNN message-passing
problem:

    x: [8192, 256] f32, attn_vectors: [4, 256] f32
    e_h = l2_normalize(attn_vectors[h] * x, axis=-1)        # [H, N, D]
    Y   = concat_h(e_h)                                     # [N, H*D]
    out = (Y @ Y.T) / H                                     # [N, N]

Strategy (8 NeuronCores, SPMD, no collectives):
  - out is symmetric: only the upper triangle is computed on-device, the host
    mirrors it.  Row blocks are dealt block-cyclically: core c owns global
    128-row blocks {8i + c}; its i-th block computes column panels [2i, 15].
    Every core runs the IDENTICAL program (k_p = p//2 + 1 blocks per panel,
    72 of the 128 baseline blocks per core).
  - Main matmul runs in fp8 e4m3 with DoubleRow perf mode (two 128-deep
    k-chunks per pass, 2x bf16 throughput, measured 216 ns per
    [K=256]x[128x512]); PSUM accumulates f32; out is written f16 and upcast
    on the host (tolerance gate is 2e-2; this pipeline measures ~1.3e-2).
  - x arrives pre-transposed/bf16 from the host (xT [256, 8192]); per-panel
    loads are plain strided DMAs, so there are no PE transposes or casts.
  - Row norms: xsq = Act Square(xT); pn[h, rows] via [4,512]-output bf16
    matmuls (w_sq stationary); rn = recip_approx_fast(sqrt(pn/S2^2)) =
    S2/||a_h*x_row||; rn bounces through DRAM and returns as a partition-
    broadcast DMA (step-0 partition APs are legal on DRAM).
  - fp8 scales: lhsT = x*a^2*r*(S1/4), rhs = x*r*S2, with S1=64, S2=16 and
    S1/(4*S2) == 1 folded into asq; the PSUM->SBUF copy applies 1/(S1*S2).
  - Elementwise work is spread: rhs panel builds on DVE + GpSimd, squares +
    sqrt + most output copies on Scalar (sqrt/copy/square share one act
    table), recip + some copies on DVE.
"""

from contextlib import ExitStack

import numpy as np

N, D, H = 8192, 256, 4
NCORES = 8
P = 128
PANEL = 512
NPANELS = N // PANEL  # 16
NBLK = 8  # local row blocks per core
CHD = D // P  # 2
KCH = H * CHD  # 8 contraction chunks of 128
NPAIR = KCH // 2  # 4 DoubleRow pairs
S1 = 64.0
S2 = 16.0
OSCALE = 1.0 / (S1 * S2)
PIPE = 4  # panels of prepass lookahead
# panels whose rhs build runs on GpSimd instead of DVE
POOL_RHS = {3, 6, 9, 12, 15}

_COMPILED = {}


def _build_bass():
    import concourse.bass as bass
    import concourse.tile as tile
    from concourse import bacc, mybir

    f32 = mybir.dt.float32
    bf16 = mybir.dt.bfloat16
    f16 = mybir.dt.float16
    fp8 = mybir.dt.float8e4
    DR = mybir.MatmulPerfMode.DoubleRow
    Sqrt = mybir.ActivationFunctionType.Sqrt
    Square = mybir.ActivationFunctionType.Square

    nc = bacc.Bacc(
        "TRN2",
        target_bir_lowering=False,
        debug=False,
        enable_asserts=False,
        num_devices=NCORES,
    )
    xT_t = nc.dram_tensor("xT", [D, N], bf16, kind="ExternalInput")
    xo_t = nc.dram_tensor("xT_own", [D, NBLK * P], bf16, kind="ExternalInput")
    ws_t = nc.dram_tensor("w_sq", [P, CHD, H], bf16, kind="ExternalInput")
    aq_t = nc.dram_tensor("asq", [P, KCH], f32, kind="ExternalInput")
    out_t = nc.dram_tensor("out", [NBLK * P, N], f16, kind="ExternalOutput")

    xTa, out = xT_t.ap(), out_t.ap()

    with tile.TileContext(nc) as tc, ExitStack() as ctx, nc.allow_low_precision(
        "fp8 kernel by design"
    ):
        consts = ctx.enter_context(tc.tile_pool(name="consts", bufs=1))
        xtp = ctx.enter_context(tc.tile_pool(name="xtp", bufs=6))
        sqp = ctx.enter_context(tc.tile_pool(name="sqp", bufs=3))
        smal = ctx.enter_context(tc.tile_pool(name="smal", bufs=3))
        bcp = ctx.enter_context(tc.tile_pool(name="bcp", bufs=5))
        rhsp = ctx.enter_context(tc.tile_pool(name="rhsp", bufs=4))
        outp = ctx.enter_context(tc.tile_pool(name="outp", bufs=2))
        tmpp = ctx.enter_context(tc.tile_pool(name="tmpp", bufs=2))
        dram = ctx.enter_context(tc.tile_pool(name="dram", bufs=1, space="DRAM"))
        ps_n = ctx.enter_context(tc.tile_pool(name="ps_n", bufs=2, space="PSUM"))
        ps_o = ctx.enter_context(tc.tile_pool(name="ps_o", bufs=5, space="PSUM"))

        w_sq = consts.tile([P, CHD, H], bf16)
        nc.sync.dma_start(w_sq[:], ws_t.ap()[:])
        asq = consts.tile([P, KCH], f32)
        nc.sync.dma_start(asq[:], aq_t.ap()[:])
        # lhsT: [128, block r, pair kk, t, m] fp8  (pair stride = 128, the
        # layout the dual-fp8 ldweights path was validated with)
        lhsT = consts.tile([P, NBLK, NPAIR, 2, P], fp8)
        xTown = consts.tile([P, CHD, NBLK, P], bf16)
        nc.sync.dma_start(
            xTown[:],
            xo_t.ap()[:].rearrange("(c q) (i n) -> q c i n", q=P, n=P),
        )

        xts = {}
        bcs = {}
        rhss = {}
        sqs = {}
        pns = {}

        def load_panel(p):
            t = xtp.tile([P, CHD, PANEL], bf16, tag="xt")
            nc.sync.dma_start(
                t[:],
                xTa[:, p * PANEL : (p + 1) * PANEL].rearrange(
                    "(c q) n -> q c n", q=P
                ),
            )
            xts[p] = t

        def square(p):
            sq = sqp.tile([P, CHD, PANEL], bf16, tag="sq")
            nc.scalar.activation(sq[:], xts[p][:], Square)
            sqs[p] = sq

        def norm_mm(p):
            pn = ps_n.tile([4, PANEL], f32, tag="pn")
            for c in range(CHD):
                nc.tensor.matmul(
                    pn[:],
                    w_sq[:, c, :],
                    sqs[p][:, c, :],
                    start=(c == 0),
                    stop=(c == CHD - 1),
                )
            pns[p] = pn

        def rnorm_bounce(p):
            s = smal.tile([4, PANEL], f32, tag="s")
            nc.scalar.activation(s[:], pns[p][:], Sqrt, scale=1.0 / (S2 * S2))
            rn = smal.tile([4, PANEL], f32, tag="rn")
            nc.vector.reciprocal_approx_fast(rn[:], s[:])
            rnd = dram.tile([4, PANEL], f32, name=f"rnd{p}")
            nc.sync.dma_start(rnd[:], rn[:])
            bc = bcp.tile([P, H, PANEL], f32, tag="bc")
            src = bass.AP(rnd.tensor, rnd.offset, [[0, P], [PANEL, H], [1, PANEL]])
            nc.sync.dma_start(bc[:], src)
            bcs[p] = bc

        def rhs_build(p):
            # rhs[q, h, c, n] = xT[q, c, n] * bc[q, h, n]
            rhs = rhsp.tile([P, NPAIR, 2, PANEL], fp8, tag="rhs")
            xt = xts[p]
            bc = bcs[p]
            in0 = bass.AP(
                xt.tensor,
                xt.offset,
                [list(xt.ap[0]), [0, H], [PANEL, CHD], [1, PANEL]],
            )
            in1 = bass.AP(
                bc.tensor,
                bc.offset,
                [list(bc.ap[0]), [PANEL, H], [0, CHD], [1, PANEL]],
            )
            eng = nc.gpsimd if p in POOL_RHS else nc.vector
            eng.tensor_tensor(rhs[:], in0, in1, mybir.AluOpType.mult)
            rhss[p] = rhs

        # ---- prologue: own-row norms + lhsT + prepass panels 0..PIPE-1 ----
        for p in range(PIPE):
            load_panel(p)

        xsq_own = consts.tile([P, CHD, NBLK, P], bf16)
        nc.scalar.activation(xsq_own[:], xTown[:], Square)
        rnd_own = dram.tile([4, NBLK * P], f32, name="rnd_own")
        for half in range(2):
            pn = ps_n.tile([4, PANEL], f32, tag="pn")
            for c in range(CHD):
                nc.tensor.matmul(
                    pn[:],
                    w_sq[:, c, :],
                    xsq_own[:, c, 4 * half : 4 * half + 4, :],
                    start=(c == 0),
                    stop=(c == CHD - 1),
                )
            s = smal.tile([4, PANEL], f32, tag="s")
            nc.scalar.activation(s[:], pn[:], Sqrt, scale=1.0 / (S2 * S2))
            rn = smal.tile([4, PANEL], f32, tag="rn")
            nc.vector.reciprocal_approx_fast(rn[:], s[:])
            nc.sync.dma_start(
                rnd_own[:, half * PANEL : (half + 1) * PANEL], rn[:]
            )
        bc_own = consts.tile([P, H, NBLK * P], f32)
        nc.sync.dma_start(
            bc_own[:],
            bass.AP(
                rnd_own.tensor,
                rnd_own.offset,
                [[0, P], [NBLK * P, H], [1, NBLK * P]],
            ),
        )

        for p in range(PIPE):
            square(p)
            norm_mm(p)
            rnorm_bounce(p)

        # lhsT build: per (h, c) and half (blocks 0-3 / 4-7):
        #   tmp = xTown * asq[kc]  (bf16, 2x tensor_scalar)
        #   lhsT[:, blocks, h, c, :] = tmp * bc_own[h]  (fp8, DVE/GpSimd)
        for half in range(2):
            blks = slice(4 * half, 4 * half + 4)
            for h in range(H):
                for c in range(CHD):
                    kc = h * CHD + c
                    tmp = tmpp.tile([P, 4, P], bf16, tag="tmp")
                    nc.vector.tensor_scalar_mul(
                        tmp[:], xTown[:, c, blks, :], asq[:, kc : kc + 1]
                    )
                    eng = nc.gpsimd if (kc % 2 == 1) else nc.vector
                    eng.tensor_tensor(
                        lhsT[:, blks, h, c, :],
                        tmp[:],
                        bc_own[:, h, half * PANEL : (half + 1) * PANEL].rearrange(
                            "q (i n) -> q i n", n=P
                        ),
                        mybir.AluOpType.mult,
                    )

        rhs_build(0)

        # ---- main loop over 16 column panels --------------------------------
        copy_cnt = 0
        for p in range(NPANELS):
            if p + PIPE < NPANELS:
                load_panel(p + PIPE)
            if p + 1 < NPANELS and (p + 1) not in POOL_RHS:
                rhs_build(p + 1)
            if p + 2 < NPANELS and (p + 2) in POOL_RHS:
                rhs_build(p + 2)

            k_p = p // 2 + 1
            rhs = rhss.pop(p)
            ot = outp.tile([P, NBLK, PANEL], f16, tag="ot")
            for r in range(k_p):
                acc = ps_o.tile([P, PANEL], f32, tag="acc")
                for kk in range(NPAIR):
                    nc.tensor.matmul(
                        acc[:],
                        lhsT[:, r, kk, :, :],
                        rhs[:, kk, :, :],
                        start=(kk == 0),
                        stop=(kk == NPAIR - 1),
                        perf_mode=DR,
                    )
                if copy_cnt % 3 == 0:
                    nc.vector.tensor_scalar_mul(ot[:, r, :], acc[:], OSCALE)
                else:
                    nc.scalar.mul(ot[:, r, :], acc[:], OSCALE)
                copy_cnt += 1
                if p == NPANELS - 1:
                    nc.sync.dma_start(
                        out[r * P : (r + 1) * P, p * PANEL : (p + 1) * PANEL],
                        ot[:, r, :],
                    )
            if p != NPANELS - 1:
                nc.sync.dma_start(
                    out[0 : k_p * P, p * PANEL : (p + 1) * PANEL].rearrange(
                        "(r q) n -> q r n", q=P
                    ),
                    ot[:, :k_p, :],
                )

            # prepass for panel p+PIPE (emitted after this panel's main work
            # so it never head-of-line blocks the consuming engines)
            pf = p + PIPE
            if pf < NPANELS:
                square(pf)
                norm_mm(pf)
                rnorm_bounce(pf)
            if p - 1 in xts:
                del xts[p - 1]
            if p in sqs:
                del sqs[p]

    nc.compile()
    return nc


def _get_compiled():
    if "nc" not in _COMPILED:
        _COMPILED["nc"] = _build_bass()
    return _COMPILED["nc"]


def host_side_inputs(x, attn):
    """Per-core input maps. Host work is layout/dtype marshaling of x plus
    tiny functions of attn_vectors; all real compute (norms, scaling, the
    N^2 matmul) runs on device."""
    import ml_dtypes

    bf16 = ml_dtypes.bfloat16
    xT = np.ascontiguousarray(x.T).astype(bf16)  # [256, 8192]
    w_sq = np.zeros((P, CHD, H), dtype=np.float32)
    asq = np.zeros((P, KCH), dtype=np.float32)
    for c in range(CHD):
        for h in range(H):
            w_sq[:, c, h] = attn[h, c * P : (c + 1) * P] ** 2
            asq[:, h * CHD + c] = attn[h, c * P : (c + 1) * P] ** 2
    w_sq = w_sq.astype(bf16)
    maps = []
    for c in range(NCORES):
        cols = np.concatenate(
            [
                np.arange((8 * i + c) * P, (8 * i + c + 1) * P)
                for i in range(NBLK)
            ]
        )
        maps.append(
            {
                "xT": xT,
                "xT_own": np.ascontiguousarray(xT[:, cols]),
                "w_sq": w_sq,
                "asq": asq,
            }
        )
    return maps


def assemble_output(results):
    """Scatter per-core row blocks into the full matrix and mirror the
    computed upper triangle."""
    full = np.empty((N, N), dtype=np.float16)
    for c in range(NCORES):
        o = results[c]["out"]
        for i in range(NBLK):
            full[(8 * i + c) * P : (8 * i + c + 1) * P, :] = o[
                i * P : (i + 1) * P, :
            ]
    f = full.astype(np.float32)
    return np.triu(f) + np.triu(f, 1).T


def kernel(**inputs) -> np.ndarray:
    from concourse import bass_utils

    x = np.ascontiguousarray(np.asarray(inputs["x"], dtype=np.float32))
    attn = np.ascontiguousarray(
        np.asarray(inputs["attn_vectors"], dtype=np.float32)
    )
    nc = _get_compiled()
    res = bass_utils.run_bass_kernel_spmd(
        nc, host_side_inputs(x, attn), core_ids=list(range(NCORES))
    )
    return assemble_output(res.results)
